# revision 25
# baseline (speedup 1.0000x reference)
"""DiffusionStep (Chebyshev K=4) GNN message passing on 8 Trainium2 cores.

v2: 1D dst-shard (6250 dst/core, padded 6272). Gather tables are fp16 with
each node's features duplicated to a 256B row ([h|h]), so int16 gather
indices cover the full 50176-row table through two overlapping views
(A=[0,32768), B=[17408,50176)); edges in the overlap zone are assigned to
whichever pass balances the per-dst split. Both passes share ONE dst order
(sorted by max(degA,degB)), so the two per-pass partial sums land in two
f32 slabs that merge with a single elementwise add — no scatter-add
re-permute. Round-0's table is assembled host-side (kills the prologue
AllGather); rounds 1-3 AllGather fp16 tables. Gather calls are 1024
descriptors (the SWDGE ring size) round-robined over 4 queues; the SWDGE
pipeline (~3ns/descriptor) is the critical resource, everything else
(DVE weight-multiply + strided reduction, collectives, output DMAs)
overlaps under it.
"""

import numpy as np

N = 50000
D = 64
NCORES = 8
SLICE = 6250
SLICE_PAD = 6272            # 49 * 128
NBLK = SLICE_PAD // 128     # 49
ROWS = NCORES * SLICE_PAD   # 50176 table rows
VA_END = 32768              # view A rows [0, 32768)
VB_START = 17408            # view B rows [17408, 50176)
H0R = 3072                  # ranks [0, 3072) -> table half 0 (blocks 0-23)
H1R = SLICE_PAD - H0R       # ranks [3072, 6272) -> half 1 (blocks 24-48)
H0 = NCORES * H0R           # 24576 rows in half 0
GCH = 64                    # staging chunks per group
GSUB = 8                    # chunks per dma_gather call (1024 descriptors)
NQ = 4                      # SWDGE queues

_cache = {}


def _raw_gather(g, out_ap, in_ap, idxs_ap, num_idxs, elem_size,
                stride_bytes_256, queue_num):
    """dma_gather with payload smaller than the 256B-granular row stride
    (bass's dma_gather asserts elem_size_bytes % 256 == 0; the descriptor
    format itself only requires the STRIDE to be 256B-granular — verified
    on hardware)."""
    import concourse.mybir as mybir
    _in_ap = g.lower_ap_dma(in_ap, for_custom_bir_dma=True)
    _idxs_ap = g.lower_ap(idxs_ap)
    _out_ap = g.lower_ap(out_ap)
    return g.add_instruction(
        mybir.InstDMAGatherAnt(
            name=g.bass.get_next_instruction_name(),
            ins=[*_in_ap, _idxs_ap, g.lower_val_access(g.to_reg(num_idxs))],
            outs=[_out_ap],
            transpose=False,
            num_idxs=num_idxs,
            elem_size=elem_size,
            stride_bytes_256=stride_bytes_256,
            gen_mode=0,
            single_packet=False,
            queue_num=queue_num,
            sbuf_tokens_per_rank=0,
            sbuf_free_dim_per_rank=0,
            sbuf_free_dim_pad_per_rank=0,
            sbuf_byte_offset=0,
        )
    )


def _gid_of(core, r):
    """Table row of (core, rank): core-major, matching the AllGather's
    concatenation layout."""
    return core * SLICE_PAD + r


def _call_sizes(n_fill, tail_small):
    """Per-group dma_gather call sizes (in chunks). The round's final group
    ends with four 2-chunk calls so the in-flight drain tail is short."""
    sizes = []
    j = 0
    while j < n_fill:
        nch = min(GSUB, n_fill - j)
        sizes.append(nch)
        j += nch
    if tail_small and sizes:
        last = sizes.pop()
        while last > 0:
            t = min(2, last)
            sizes.append(t)
            last -= t
    return sizes


def _build_structure(edge_index, edge_weight):
    src = edge_index[0].astype(np.int64)
    dst = edge_index[1].astype(np.int64)
    w = edge_weight.astype(np.float32)

    node_core = np.arange(N) // SLICE
    node_local = np.arange(N) - node_core * SLICE
    dcore = dst // SLICE
    dloc = dst - dcore * SLICE

    deg_t = np.zeros((NCORES, SLICE_PAD), np.int64)
    for c in range(NCORES):
        deg_t[c] = np.bincount(dloc[dcore == c], minlength=SLICE_PAD)

    def build_rank(key):
        rank = np.zeros((NCORES, SLICE_PAD), np.int64)
        pi = np.zeros((NCORES, SLICE_PAD), np.int64)
        for c in range(NCORES):
            order = np.argsort(key[c], kind="stable")
            pi[c] = order
            rank[c, order] = np.arange(SLICE_PAD)
        return rank, pi

    rank, pi = build_rank(deg_t)
    e_pass_c = {}
    # 3 balance passes; re-rank (by max per-pass degree) after the first two.
    # The final balance runs against the FINAL rank so pass assignments and
    # idx zones are consistent.
    for it in range(3):
        gid = _gid_of(node_core, rank[node_core, node_local])
        sgid = gid[src]
        passf = np.where(sgid < VB_START, 0, np.where(sgid >= VA_END, 1, -1))
        key = np.zeros((NCORES, SLICE_PAD), np.int64)
        for c in range(NCORES):
            m = np.nonzero(dcore == c)[0]
            e_dst = dloc[m]
            e_pass = passf[m].copy()
            fA = np.bincount(e_dst[e_pass == 0], minlength=SLICE_PAD)
            fB = np.bincount(e_dst[e_pass == 1], minlength=SLICE_PAD)
            flex = np.nonzero(e_pass == -1)[0]
            fd = e_dst[flex]
            F = np.bincount(fd, minlength=SLICE_PAD)
            tot = fA + fB + F
            nA = np.clip((tot + 1) // 2, fA, fA + F) - fA  # extra A per dst
            order = np.argsort(fd, kind="stable")
            sf = flex[order]
            starts = np.concatenate([[0], np.cumsum(F)[:-1]])
            within = np.arange(len(sf)) - starts[fd[order]]
            e_pass[sf] = np.where(within < nA[fd[order]], 0, 1)
            degA = np.bincount(e_dst[e_pass == 0], minlength=SLICE_PAD)
            degB = np.bincount(e_dst[e_pass == 1], minlength=SLICE_PAD)
            key[c] = np.maximum(degA, degB)
            e_pass_c[c] = (m, e_pass, degA, degB)
        if it < 2:
            rank, pi = build_rank(key)

    gid = _gid_of(node_core, rank[node_core, node_local])
    sgid = gid[src]

    # per-pass block max degree with the SHARED final order
    R = np.zeros((NCORES, 2, NBLK), np.int64)
    for c in range(NCORES):
        _, _, degA, degB = e_pass_c[c]
        R[c, 0] = np.maximum(degA[pi[c]].reshape(NBLK, 128).max(1), 1)
        R[c, 1] = np.maximum(degB[pi[c]].reshape(NBLK, 128).max(1), 1)
    Rmax = R.max(axis=0)  # shared group shapes across cores: [2, NBLK]

    # group packing (shared across cores): first-fit-decreasing per pass
    groups = []      # (pass, chunk_base, n_fill, [(b, off_in_group)])
    chunk_base = 0
    blk_chunk = np.zeros((2, NBLK), np.int64)
    for p in range(2):
        order = sorted(range(NBLK), key=lambda b: -int(Rmax[p, b]))
        bins = []
        for b in order:
            rb = int(Rmax[p, b])
            placed = False
            for bin_ in bins:
                if bin_[0] + rb <= GCH:
                    bin_[1].append((b, bin_[0]))
                    bin_[0] += rb
                    placed = True
                    break
            if not placed:
                bins.append([rb, [(b, 0)]])
        bins = sorted(bins, key=lambda b_: -b_[0])
        if len(bins) > 1 and len(bins[-1][1]) > 1:
            # peel the smallest block off the smallest bin so the round's
            # final group is a single small block (short DVE/drain tail)
            fill, blist = bins.pop()
            bs = sorted((b for b, _ in blist), key=lambda b: Rmax[p, b])
            tailb = bs[0]
            rest, acc = [], 0
            for b in bs[1:][::-1]:
                rest.append((b, acc))
                acc += int(Rmax[p, b])
            bins.append([acc, rest])
            bins.append([int(Rmax[p, tailb]), [(tailb, 0)]])
        for fill, blist in bins:
            groups.append((p, chunk_base, fill, blist))
            chunk_base += fill
    TOT = chunk_base
    for (p, base, n_fill, blist) in groups:
        for (b, off) in blist:
            blk_chunk[p, b] = base + off

    idx_all = np.zeros((NCORES, TOT * 128), np.int64)
    w_all = np.zeros((NCORES, 128, TOT), np.float16)

    for c in range(NCORES):
        m, e_pass, degA, degB = e_pass_c[c]
        d_loc = dloc[m]
        for p in range(2):
            sel = m[e_pass == p]
            dl = d_loc[e_pass == p]
            r_of_dst = rank[c, dl]
            # secondary sort by src gid: each dst's slots read ascending
            # addresses, improving HBM row locality during the drain
            order = np.lexsort((sgid[sel], r_of_dst))
            sel = sel[order]
            d_rank = r_of_dst[order]
            counts = np.bincount(d_rank, minlength=SLICE_PAD)
            starts = np.concatenate([[0], np.cumsum(counts)[:-1]])
            within = np.arange(len(sel)) - starts[d_rank]
            b_of = d_rank // 128
            chunk = blk_chunk[p, b_of] + within
            slot = chunk * 128 + (d_rank % 128)
            g = sgid[sel]
            idx_all[c, slot] = g - (0 if p == 0 else VB_START)
            w_all[c, slot % 128, slot // 128] = w[sel]

    # wrap indices per call (GSUB chunks = 1024 descriptors), replicate to
    # 128 partitions host-side
    idx_wrapped = np.zeros((NCORES, 128, TOT * 8), np.int16)
    for c in range(NCORES):
        w16 = np.zeros((16, TOT * 8), np.int16)
        for gi, (p, base, n_fill, blist) in enumerate(groups):
            j = 0
            for nch in _call_sizes(n_fill, gi == len(groups) - 1):
                a = idx_all[c, (base + j) * 128:(base + j + nch) * 128]
                wrp = a.reshape(-1, 16).T.astype(np.uint16).view(np.int16)
                w16[:, (base + j) * 8:(base + j + nch) * 8] = wrp
                j += nch
        idx_wrapped[c] = np.tile(w16, (8, 1))

    meta = dict(R=Rmax, TOT=TOT, groups=groups, rank=rank, gid=gid,
                node_core=node_core, node_local=node_local)
    return meta, idx_wrapped, w_all


def _build_program(meta, timing_rep=0):
    import concourse.bacc as bacc
    import concourse.mybir as mybir
    import concourse.tile as tile

    R = meta["R"]
    TOT = meta["TOT"]
    groups = meta["groups"]

    nc = bacc.Bacc(num_devices=NCORES, num_swdge_queues=NQ)
    f32 = mybir.dt.float32
    f16 = mybir.dt.float16

    x_tab = nc.dram_tensor("x_tab", [ROWS, 128], f16, kind="ExternalInput")
    x_sl = nc.dram_tensor("x_sl", [SLICE_PAD, D], f32, kind="ExternalInput")
    idx_in = nc.dram_tensor("idx", [128, TOT * 8], mybir.dt.int16,
                            kind="ExternalInput")
    w_in = nc.dram_tensor("w", [128, TOT], f16, kind="ExternalInput")
    out_t = nc.dram_tensor("out", [4, SLICE_PAD, D], f32,
                           kind="ExternalOutput")
    ag_in = [nc.dram_tensor(f"agin{k}", [SLICE_PAD, 128], f16,
                            kind="Internal") for k in range(3)]
    h_tab = [nc.dram_tensor(f"htab{k}", [ROWS, 128], f16, kind="Internal",
                            addr_space="Shared") for k in range(3)]

    SLAB = NBLK * D

    with tile.TileContext(nc) as tc:
        with (
            tc.tile_pool(name="const", bufs=1) as constp,
            tc.tile_pool(name="stg", bufs=10) as stgp,
        ):
            idx_sb = constp.tile([128, TOT * 8], mybir.dt.int16,
                                 name="idx_sb")
            w_sb = constp.tile([128, TOT], f16, name="w_sb")
            # split the idx prologue so the first gather calls start as soon
            # as their columns land
            CUT = 128 * 8
            nc.sync.dma_start(idx_sb[:, 0:CUT], idx_in.ap()[:, 0:CUT])
            nc.sync.dma_start(idx_sb[:, CUT:], idx_in.ap()[:, CUT:])
            nc.sync.dma_start(w_sb[:], w_in.ap())

            accA = constp.tile([128, SLAB], f32, name="accA")
            accB = constp.tile([128, SLAB], f32, name="accB")
            acc = [accA, accB]
            t16 = constp.tile([128, SLAB], f16, name="t16")
            S0 = constp.tile([128, SLAB], f32, name="S0")
            S1 = constp.tile([128, SLAB], f32, name="S1")
            S2 = constp.tile([128, SLAB], f32, name="S2")

            def sl_dram(t, k=None):
                ap = t.ap() if k is None else t.ap()[k, :, :]
                return ap.rearrange("(b p) d -> p b d", p=128)

            def sb3(t):
                return t[:].rearrange("p (b d) -> p b d", d=D)

            nc.sync.dma_start(sb3(S0),
                              x_sl.ap().rearrange("(b p) d -> p b d", p=128))

            qn = [0]

            def next_q():
                qn[0] = (qn[0] + 1) % NQ
                return qn[0]

            rep_cm = tc.For_i(0, timing_rep, 1) if timing_rep > 1 else None
            if rep_cm is not None:
                rep_cm.__enter__()
            Tm1, Tm2, Tcur = S0, S1, S2
            for k in range(4):
                src_tab = (x_tab.ap() if (k == 0 or timing_rep)
                           else h_tab[k - 1].ap())
                views = [src_tab[0:VA_END, 0:D],
                         src_tab[VB_START:ROWS, 0:D]]
                for gi, (p, base, n_fill, blist) in enumerate(groups):
                    sgt = stgp.tile([128, GCH, D], f16, tag="sg",
                                    name=f"sg_{k}_{p}_{base}")
                    j = 0
                    for nch in _call_sizes(n_fill, gi == len(groups) - 1):
                        _raw_gather(
                            nc.gpsimd, sgt[:, j:j + nch, :], views[p],
                            idx_sb[:, (base + j) * 8:(base + j + nch) * 8],
                            nch * 128, D, 1, next_q())
                        j += nch
                    wv = w_sb[:, base:base + n_fill, None].to_broadcast(
                        [128, n_fill, D])
                    nc.vector.tensor_tensor(
                        out=sgt[:, 0:n_fill, 0:D], in0=sgt[:, 0:n_fill, 0:D],
                        in1=wv, op=mybir.AluOpType.mult)
                    # merge runs of consecutive equal-R blocks into one reduce
                    runs = []
                    for (b, off) in blist:
                        rb = int(R[p, b])
                        if (runs and runs[-1][2] == rb
                                and b == runs[-1][0] + runs[-1][3]
                                and off == runs[-1][1] + runs[-1][3] * rb):
                            runs[-1][3] += 1
                        else:
                            runs.append([b, off, rb, 1])
                    for (b0, off0, rb, nb) in runs:
                        inap = sgt[:, off0:off0 + nb * rb, 0:D].rearrange(
                            "p (n r) d -> p n d r", r=rb)
                        nc.vector.tensor_reduce(
                            out=acc[p][:, b0 * D:(b0 + nb) * D],
                            in_=inap, axis=mybir.AxisListType.X,
                            op=mybir.AluOpType.add)

                # combine + Chebyshev + cast + table-slice DMA in block-range
                # chunks so the round tail is short (each chunk only waits on
                # the reduces covering its block range)
                # quarter-chunks aligned to halves: [0,12,24) -> half 0,
                # [24,37,49) -> half 1. Emit in readiness order; emit each
                # half's AllGather as soon as both its quarters are written.
                HALVES = [(0, 24), (24, NBLK)]
                QS = [(0, 12), (12, 24), (24, 37), (37, NBLK)]
                last_g = {}
                for gi, (p, _, _, blist) in enumerate(groups):
                    for (b, _) in blist:
                        last_g[b] = max(last_g.get(b, 0), gi)
                QS = sorted(QS, key=lambda q: max(
                    last_g.get(b, 0) for b in range(q[0], q[1])))
                done_q = set()

                def maybe_collective(k):
                    need = {(0, 12), (12, 24), (24, 37), (37, NBLK)}
                    if (k, "done") in done_q or not need <= done_q:
                        return
                    done_q.add((k, "done"))
                    nc.gpsimd.collective_compute(
                        "AllGather",
                        mybir.AluOpType.bypass,
                        replica_groups=[list(range(NCORES))],
                        ins=[ag_in[k].ap()],
                        outs=[h_tab[k].ap()],
                    )

                for (b0, b1) in QS:
                    r0, r1 = b0 * D, b1 * D
                    nc.vector.tensor_tensor(
                        out=accA[:, r0:r1], in0=accA[:, r0:r1],
                        in1=accB[:, r0:r1], op=mybir.AluOpType.add)
                    if k == 0:
                        nc.vector.tensor_tensor(
                            out=Tcur[:, r0:r1], in0=Tm1[:, r0:r1],
                            in1=accA[:, r0:r1], op=mybir.AluOpType.subtract)
                    else:
                        nc.vector.tensor_tensor(
                            out=accA[:, r0:r1], in0=Tm1[:, r0:r1],
                            in1=accA[:, r0:r1], op=mybir.AluOpType.subtract)
                        nc.vector.scalar_tensor_tensor(
                            out=Tcur[:, r0:r1], in0=accA[:, r0:r1],
                            scalar=2.0, in1=Tm2[:, r0:r1],
                            op0=mybir.AluOpType.mult,
                            op1=mybir.AluOpType.subtract)
                    if k < 3:
                        nc.vector.tensor_copy(out=t16[:, r0:r1],
                                              in_=Tcur[:, r0:r1])
                        tv = t16[:, r0:r1].rearrange("p (b d) -> p b d", d=D)
                        nc.sync.dma_start(
                            ag_in[k].ap()[b0 * 128:b1 * 128, 0:D].rearrange(
                                "(b p) d -> p b d", p=128), tv)
                        done_q.add((b0, b1))
                        if not timing_rep:
                            maybe_collective(k)
                nc.sync.dma_start(sl_dram(out_t, k), sb3(Tcur))
                if k < 3:
                    Tm1, Tm2, Tcur = Tcur, Tm1, Tm2
            if rep_cm is not None:
                rep_cm.__exit__(None, None, None)

    nc.compile()
    return nc


def kernel(x, edge_index, edge_weight):
    from concourse.bass_utils import run_bass_kernel_spmd

    x = np.asarray(x, dtype=np.float32)
    import hashlib
    ei_b = np.ascontiguousarray(edge_index)
    ew_b = np.ascontiguousarray(edge_weight)
    key = hashlib.md5(ei_b.tobytes() + ew_b.tobytes()).hexdigest()
    if _cache.get("key") != key:
        _cache.clear()
        _cache["key"] = key
        ei = np.asarray(edge_index)
        ew = np.asarray(edge_weight, dtype=np.float32)
        meta, idx_w, w_all = _build_structure(ei, ew)
        nc = _build_program(meta)
        _cache["built"] = (meta, idx_w, w_all, nc)
    meta, idx_w, w_all, nc = _cache["built"]

    gid = meta["gid"]
    rank = meta["rank"]
    node_core = meta["node_core"]
    node_local = meta["node_local"]

    x16 = x.astype(np.float16)
    x_tab = np.zeros((ROWS, 128), np.float16)
    x_tab[gid, 0:D] = x16
    x_sl_all = np.zeros((NCORES, SLICE_PAD, D), np.float32)
    for c in range(NCORES):
        nodes = np.nonzero(node_core == c)[0]
        x_sl_all[c, rank[c, node_local[nodes]]] = x[nodes]

    in_maps = []
    for c in range(NCORES):
        in_maps.append({
            "x_tab": x_tab,
            "x_sl": x_sl_all[c],
            "idx": idx_w[c],
            "w": w_all[c],
        })
    try:
        res = run_bass_kernel_spmd(nc, in_maps, core_ids=list(range(NCORES)))
    except Exception:
        import time as _t
        _t.sleep(2.0)
        res = run_bass_kernel_spmd(nc, in_maps, core_ids=list(range(NCORES)))

    out = np.empty((5, N, D), np.float32)
    out[0] = x
    for c in range(NCORES):
        o = res.results[c]["out"]
        nodes = np.nonzero(node_core == c)[0]
        rk = rank[c, node_local[nodes]]
        out[1:, nodes, :] = o[:, rk, :]
    return out


# revision 26
# speedup vs baseline: 1.0279x; 1.0279x over previous
"""DiffusionStep (Chebyshev K=4) GNN message passing on 8 Trainium2 cores.

1D dst-shard (6250 dst/core, padded 6272). Per propagate, each core SWDGE-
gathers its in-edges' source rows from a DRAM table: fp16 rows on a 256B
stride with a 128B payload (elem_size < stride; bass's %256 assert is
bypassed via direct instruction construction — hardware-verified). int16
indices cover the 50176-row table through two overlapping views
(A=[0,32768), B=[17408,50176)); overlap-zone edges go to whichever pass
balances the per-dst split. Both passes share ONE dst order (sorted by
max(degA,degB)) so the per-pass partial sums merge with an elementwise add
— no scatter-add re-permute. Within each dst, slots are sorted by source
address (HBM row locality — worth ~25% of drain time). Gather calls are
1024 descriptors (the SWDGE ring size) round-robined over 4 queues at
~2.6ns/descriptor, the machine's critical resource; DVE weight-multiply +
strided reduction, fp16 AllGathers (round-0's table is host-assembled),
and output DMAs all overlap under it. Round tails are trimmed: the last
group is a single small block, the final calls are 2-chunk, and the
combine/cast/table-write chain is emitted in block-range quarters ordered
by readiness with the AllGather fired as soon as the last quarter lands.
"""

import numpy as np

N = 50000
D = 64
NCORES = 8
SLICE = 6250
SLICE_PAD = 6272            # 49 * 128
NBLK = SLICE_PAD // 128     # 49
ROWS = NCORES * SLICE_PAD   # 50176 table rows
VA_END = 32768              # view A rows [0, 32768)
VB_START = 17408            # view B rows [17408, 50176)
H0R = 3072                  # ranks [0, 3072) -> table half 0 (blocks 0-23)
H1R = SLICE_PAD - H0R       # ranks [3072, 6272) -> half 1 (blocks 24-48)
H0 = NCORES * H0R           # 24576 rows in half 0
GCH = 64                    # staging chunks per group
GSUB = 8                    # chunks per dma_gather call (1024 descriptors)
NQ = 4                      # SWDGE queues

_cache = {}


def _raw_gather(g, out_ap, in_ap, idxs_ap, num_idxs, elem_size,
                stride_bytes_256, queue_num):
    """dma_gather with payload smaller than the 256B-granular row stride
    (bass's dma_gather asserts elem_size_bytes % 256 == 0; the descriptor
    format itself only requires the STRIDE to be 256B-granular — verified
    on hardware)."""
    import concourse.mybir as mybir
    _in_ap = g.lower_ap_dma(in_ap, for_custom_bir_dma=True)
    _idxs_ap = g.lower_ap(idxs_ap)
    _out_ap = g.lower_ap(out_ap)
    return g.add_instruction(
        mybir.InstDMAGatherAnt(
            name=g.bass.get_next_instruction_name(),
            ins=[*_in_ap, _idxs_ap, g.lower_val_access(g.to_reg(num_idxs))],
            outs=[_out_ap],
            transpose=False,
            num_idxs=num_idxs,
            elem_size=elem_size,
            stride_bytes_256=stride_bytes_256,
            gen_mode=0,
            single_packet=False,
            queue_num=queue_num,
            sbuf_tokens_per_rank=0,
            sbuf_free_dim_per_rank=0,
            sbuf_free_dim_pad_per_rank=0,
            sbuf_byte_offset=0,
        )
    )


def _gid_of(core, r):
    """Table row of (core, rank): core-major, matching the AllGather's
    concatenation layout."""
    return core * SLICE_PAD + r


def _call_sizes(n_fill, tail_small):
    """Per-group dma_gather call sizes (in chunks). The round's final group
    ends with four 2-chunk calls so the in-flight drain tail is short."""
    sizes = []
    j = 0
    while j < n_fill:
        nch = min(GSUB, n_fill - j)
        sizes.append(nch)
        j += nch
    if tail_small and sizes:
        last = sizes.pop()
        while last > 0:
            t = min(2, last)
            sizes.append(t)
            last -= t
    return sizes


def _build_structure(edge_index, edge_weight):
    src = edge_index[0].astype(np.int64)
    dst = edge_index[1].astype(np.int64)
    w = edge_weight.astype(np.float32)

    node_core = np.arange(N) // SLICE
    node_local = np.arange(N) - node_core * SLICE
    dcore = dst // SLICE
    dloc = dst - dcore * SLICE

    deg_t = np.zeros((NCORES, SLICE_PAD), np.int64)
    for c in range(NCORES):
        deg_t[c] = np.bincount(dloc[dcore == c], minlength=SLICE_PAD)

    def build_rank(key):
        rank = np.zeros((NCORES, SLICE_PAD), np.int64)
        pi = np.zeros((NCORES, SLICE_PAD), np.int64)
        for c in range(NCORES):
            order = np.argsort(key[c], kind="stable")
            pi[c] = order
            rank[c, order] = np.arange(SLICE_PAD)
        return rank, pi

    rank, pi = build_rank(deg_t)
    e_pass_c = {}
    # 3 balance passes; re-rank (by max per-pass degree) after the first two.
    # The final balance runs against the FINAL rank so pass assignments and
    # idx zones are consistent.
    for it in range(3):
        gid = _gid_of(node_core, rank[node_core, node_local])
        sgid = gid[src]
        passf = np.where(sgid < VB_START, 0, np.where(sgid >= VA_END, 1, -1))
        key = np.zeros((NCORES, SLICE_PAD), np.int64)
        for c in range(NCORES):
            m = np.nonzero(dcore == c)[0]
            e_dst = dloc[m]
            e_pass = passf[m].copy()
            fA = np.bincount(e_dst[e_pass == 0], minlength=SLICE_PAD)
            fB = np.bincount(e_dst[e_pass == 1], minlength=SLICE_PAD)
            flex = np.nonzero(e_pass == -1)[0]
            fd = e_dst[flex]
            F = np.bincount(fd, minlength=SLICE_PAD)
            tot = fA + fB + F
            nA = np.clip((tot + 1) // 2, fA, fA + F) - fA  # extra A per dst
            order = np.argsort(fd, kind="stable")
            sf = flex[order]
            starts = np.concatenate([[0], np.cumsum(F)[:-1]])
            within = np.arange(len(sf)) - starts[fd[order]]
            e_pass[sf] = np.where(within < nA[fd[order]], 0, 1)
            degA = np.bincount(e_dst[e_pass == 0], minlength=SLICE_PAD)
            degB = np.bincount(e_dst[e_pass == 1], minlength=SLICE_PAD)
            key[c] = np.maximum(degA, degB)
            e_pass_c[c] = (m, e_pass, degA, degB)
        if it < 2:
            rank, pi = build_rank(key)

    gid = _gid_of(node_core, rank[node_core, node_local])
    sgid = gid[src]

    # per-pass block max degree with the SHARED final order
    R = np.zeros((NCORES, 2, NBLK), np.int64)
    for c in range(NCORES):
        _, _, degA, degB = e_pass_c[c]
        R[c, 0] = np.maximum(degA[pi[c]].reshape(NBLK, 128).max(1), 1)
        R[c, 1] = np.maximum(degB[pi[c]].reshape(NBLK, 128).max(1), 1)
    Rmax = R.max(axis=0)  # shared group shapes across cores: [2, NBLK]

    # group packing (shared across cores): first-fit-decreasing per pass
    groups = []      # (pass, chunk_base, n_fill, [(b, off_in_group)])
    chunk_base = 0
    blk_chunk = np.zeros((2, NBLK), np.int64)
    for p in range(2):
        order = sorted(range(NBLK), key=lambda b: -int(Rmax[p, b]))
        bins = []
        for b in order:
            rb = int(Rmax[p, b])
            placed = False
            for bin_ in bins:
                if bin_[0] + rb <= GCH:
                    bin_[1].append((b, bin_[0]))
                    bin_[0] += rb
                    placed = True
                    break
            if not placed:
                bins.append([rb, [(b, 0)]])
        bins = sorted(bins, key=lambda b_: -b_[0])
        if len(bins) > 1 and len(bins[-1][1]) > 1:
            # peel the smallest block off the smallest bin so the round's
            # final group is a single small block (short DVE/drain tail)
            fill, blist = bins.pop()
            bs = sorted((b for b, _ in blist), key=lambda b: Rmax[p, b])
            tailb = bs[0]
            rest, acc = [], 0
            for b in bs[1:][::-1]:
                rest.append((b, acc))
                acc += int(Rmax[p, b])
            bins.append([acc, rest])
            bins.append([int(Rmax[p, tailb]), [(tailb, 0)]])
        for fill, blist in bins:
            groups.append((p, chunk_base, fill, blist))
            chunk_base += fill
    TOT = chunk_base
    for (p, base, n_fill, blist) in groups:
        for (b, off) in blist:
            blk_chunk[p, b] = base + off

    idx_all = np.zeros((NCORES, TOT * 128), np.int64)
    w_all = np.zeros((NCORES, 128, TOT), np.float16)

    for c in range(NCORES):
        m, e_pass, degA, degB = e_pass_c[c]
        d_loc = dloc[m]
        for p in range(2):
            sel = m[e_pass == p]
            dl = d_loc[e_pass == p]
            r_of_dst = rank[c, dl]
            # secondary sort by src gid: each dst's slots read ascending
            # addresses, improving HBM row locality during the drain
            order = np.lexsort((sgid[sel], r_of_dst))
            sel = sel[order]
            d_rank = r_of_dst[order]
            counts = np.bincount(d_rank, minlength=SLICE_PAD)
            starts = np.concatenate([[0], np.cumsum(counts)[:-1]])
            within = np.arange(len(sel)) - starts[d_rank]
            b_of = d_rank // 128
            chunk = blk_chunk[p, b_of] + within
            slot = chunk * 128 + (d_rank % 128)
            g = sgid[sel]
            idx_all[c, slot] = g - (0 if p == 0 else VB_START)
            w_all[c, slot % 128, slot // 128] = w[sel]

    # wrap indices per call (GSUB chunks = 1024 descriptors), replicate to
    # 128 partitions host-side
    idx_wrapped = np.zeros((NCORES, 128, TOT * 8), np.int16)
    for c in range(NCORES):
        w16 = np.zeros((16, TOT * 8), np.int16)
        for gi, (p, base, n_fill, blist) in enumerate(groups):
            j = 0
            for nch in _call_sizes(n_fill, gi == len(groups) - 1):
                a = idx_all[c, (base + j) * 128:(base + j + nch) * 128]
                wrp = a.reshape(-1, 16).T.astype(np.uint16).view(np.int16)
                w16[:, (base + j) * 8:(base + j + nch) * 8] = wrp
                j += nch
        idx_wrapped[c] = np.tile(w16, (8, 1))

    meta = dict(R=Rmax, TOT=TOT, groups=groups, rank=rank, gid=gid,
                node_core=node_core, node_local=node_local)
    return meta, idx_wrapped, w_all


def _build_program(meta, timing_rep=0):
    import concourse.bacc as bacc
    import concourse.mybir as mybir
    import concourse.tile as tile

    R = meta["R"]
    TOT = meta["TOT"]
    groups = meta["groups"]

    nc = bacc.Bacc(num_devices=NCORES, num_swdge_queues=NQ)
    f32 = mybir.dt.float32
    f16 = mybir.dt.float16

    x_tab = nc.dram_tensor("x_tab", [ROWS, 128], f16, kind="ExternalInput")
    x_sl = nc.dram_tensor("x_sl", [SLICE_PAD, D], f32, kind="ExternalInput")
    idx_in = nc.dram_tensor("idx", [128, TOT * 8], mybir.dt.int16,
                            kind="ExternalInput")
    w_in = nc.dram_tensor("w", [128, TOT], f16, kind="ExternalInput")
    out_t = nc.dram_tensor("out", [4, SLICE_PAD, D], f32,
                           kind="ExternalOutput")
    ag_in = [nc.dram_tensor(f"agin{k}", [SLICE_PAD, 128], f16,
                            kind="Internal") for k in range(3)]
    h_tab = [nc.dram_tensor(f"htab{k}", [ROWS, 128], f16, kind="Internal",
                            addr_space="Shared") for k in range(3)]

    SLAB = NBLK * D

    with tile.TileContext(nc) as tc:
        with (
            tc.tile_pool(name="const", bufs=1) as constp,
            tc.tile_pool(name="stg", bufs=10) as stgp,
        ):
            idx_sb = constp.tile([128, TOT * 8], mybir.dt.int16,
                                 name="idx_sb")
            w_sb = constp.tile([128, TOT], f16, name="w_sb")
            # split the idx prologue so the first gather calls start as soon
            # as their columns land
            CUT = 128 * 8
            nc.sync.dma_start(idx_sb[:, 0:CUT], idx_in.ap()[:, 0:CUT])
            nc.sync.dma_start(idx_sb[:, CUT:], idx_in.ap()[:, CUT:])
            nc.sync.dma_start(w_sb[:], w_in.ap())

            accA = constp.tile([128, SLAB], f32, name="accA")
            accB = constp.tile([128, SLAB], f32, name="accB")
            acc = [accA, accB]
            t16 = constp.tile([128, SLAB], f16, name="t16")
            S0 = constp.tile([128, SLAB], f32, name="S0")
            S1 = constp.tile([128, SLAB], f32, name="S1")
            S2 = constp.tile([128, SLAB], f32, name="S2")

            def sl_dram(t, k=None):
                ap = t.ap() if k is None else t.ap()[k, :, :]
                return ap.rearrange("(b p) d -> p b d", p=128)

            def sb3(t):
                return t[:].rearrange("p (b d) -> p b d", d=D)

            nc.sync.dma_start(sb3(S0),
                              x_sl.ap().rearrange("(b p) d -> p b d", p=128))

            qn = [0]

            def next_q():
                qn[0] = (qn[0] + 1) % NQ
                return qn[0]

            rep_cm = tc.For_i(0, timing_rep, 1) if timing_rep > 1 else None
            if rep_cm is not None:
                rep_cm.__enter__()
            Tm1, Tm2, Tcur = S0, S1, S2
            for k in range(4):
                src_tab = (x_tab.ap() if (k == 0 or timing_rep)
                           else h_tab[k - 1].ap())
                views = [src_tab[0:VA_END, 0:D],
                         src_tab[VB_START:ROWS, 0:D]]
                for gi, (p, base, n_fill, blist) in enumerate(groups):
                    sgt = stgp.tile([128, GCH, D], f16, tag="sg",
                                    name=f"sg_{k}_{p}_{base}")
                    j = 0
                    for nch in _call_sizes(n_fill, gi == len(groups) - 1):
                        _raw_gather(
                            nc.gpsimd, sgt[:, j:j + nch, :], views[p],
                            idx_sb[:, (base + j) * 8:(base + j + nch) * 8],
                            nch * 128, D, 1, next_q())
                        j += nch
                    wv = w_sb[:, base:base + n_fill, None].to_broadcast(
                        [128, n_fill, D])
                    nc.vector.tensor_tensor(
                        out=sgt[:, 0:n_fill, 0:D], in0=sgt[:, 0:n_fill, 0:D],
                        in1=wv, op=mybir.AluOpType.mult)
                    # merge runs of consecutive equal-R blocks into one reduce
                    runs = []
                    for (b, off) in blist:
                        rb = int(R[p, b])
                        if (runs and runs[-1][2] == rb
                                and b == runs[-1][0] + runs[-1][3]
                                and off == runs[-1][1] + runs[-1][3] * rb):
                            runs[-1][3] += 1
                        else:
                            runs.append([b, off, rb, 1])
                    for (b0, off0, rb, nb) in runs:
                        inap = sgt[:, off0:off0 + nb * rb, 0:D].rearrange(
                            "p (n r) d -> p n d r", r=rb)
                        nc.vector.tensor_reduce(
                            out=acc[p][:, b0 * D:(b0 + nb) * D],
                            in_=inap, axis=mybir.AxisListType.X,
                            op=mybir.AluOpType.add)

                # combine + Chebyshev + cast + table-slice DMA in block-range
                # chunks so the round tail is short (each chunk only waits on
                # the reduces covering its block range)
                # quarter-chunks aligned to halves: [0,12,24) -> half 0,
                # [24,37,49) -> half 1. Emit in readiness order; emit each
                # half's AllGather as soon as both its quarters are written.
                QS = [(0, 12), (12, 24), (24, 37), (37, NBLK)]
                last_g = {}
                for gi, (p, _, _, blist) in enumerate(groups):
                    for (b, _) in blist:
                        last_g[b] = max(last_g.get(b, 0), gi)
                QS = sorted(QS, key=lambda q: max(
                    last_g.get(b, 0) for b in range(q[0], q[1])))
                done_q = set()

                def maybe_collective(k):
                    need = {(0, 12), (12, 24), (24, 37), (37, NBLK)}
                    if (k, "done") in done_q or not need <= done_q:
                        return
                    done_q.add((k, "done"))
                    nc.gpsimd.collective_compute(
                        "AllGather",
                        mybir.AluOpType.bypass,
                        replica_groups=[list(range(NCORES))],
                        ins=[ag_in[k].ap()],
                        outs=[h_tab[k].ap()],
                    )

                for (b0, b1) in QS:
                    r0, r1 = b0 * D, b1 * D
                    nc.vector.tensor_tensor(
                        out=accA[:, r0:r1], in0=accA[:, r0:r1],
                        in1=accB[:, r0:r1], op=mybir.AluOpType.add)
                    if k == 0:
                        nc.vector.tensor_tensor(
                            out=Tcur[:, r0:r1], in0=Tm1[:, r0:r1],
                            in1=accA[:, r0:r1], op=mybir.AluOpType.subtract)
                    else:
                        nc.vector.tensor_tensor(
                            out=accA[:, r0:r1], in0=Tm1[:, r0:r1],
                            in1=accA[:, r0:r1], op=mybir.AluOpType.subtract)
                        nc.vector.scalar_tensor_tensor(
                            out=Tcur[:, r0:r1], in0=accA[:, r0:r1],
                            scalar=2.0, in1=Tm2[:, r0:r1],
                            op0=mybir.AluOpType.mult,
                            op1=mybir.AluOpType.subtract)
                    if k < 3:
                        nc.vector.tensor_copy(out=t16[:, r0:r1],
                                              in_=Tcur[:, r0:r1])
                        tv = t16[:, r0:r1].rearrange("p (b d) -> p b d", d=D)
                        nc.sync.dma_start(
                            ag_in[k].ap()[b0 * 128:b1 * 128, 0:D].rearrange(
                                "(b p) d -> p b d", p=128), tv)
                        done_q.add((b0, b1))
                        if not timing_rep:
                            maybe_collective(k)
                nc.sync.dma_start(sl_dram(out_t, k), sb3(Tcur))
                if k < 3:
                    Tm1, Tm2, Tcur = Tcur, Tm1, Tm2
            if rep_cm is not None:
                rep_cm.__exit__(None, None, None)

    nc.compile()
    return nc


def kernel(x, edge_index, edge_weight):
    from concourse.bass_utils import run_bass_kernel_spmd

    x = np.asarray(x, dtype=np.float32)
    import hashlib
    ei_b = np.ascontiguousarray(edge_index)
    ew_b = np.ascontiguousarray(edge_weight)
    key = hashlib.md5(ei_b.tobytes() + ew_b.tobytes()).hexdigest()
    if _cache.get("key") != key:
        _cache.clear()
        _cache["key"] = key
        ei = np.asarray(edge_index)
        ew = np.asarray(edge_weight, dtype=np.float32)
        meta, idx_w, w_all = _build_structure(ei, ew)
        nc = _build_program(meta)
        _cache["built"] = (meta, idx_w, w_all, nc)
    meta, idx_w, w_all, nc = _cache["built"]

    gid = meta["gid"]
    rank = meta["rank"]
    node_core = meta["node_core"]
    node_local = meta["node_local"]

    x16 = x.astype(np.float16)
    x_tab = np.zeros((ROWS, 128), np.float16)
    x_tab[gid, 0:D] = x16
    x_sl_all = np.zeros((NCORES, SLICE_PAD, D), np.float32)
    for c in range(NCORES):
        nodes = np.nonzero(node_core == c)[0]
        x_sl_all[c, rank[c, node_local[nodes]]] = x[nodes]

    in_maps = []
    for c in range(NCORES):
        in_maps.append({
            "x_tab": x_tab,
            "x_sl": x_sl_all[c],
            "idx": idx_w[c],
            "w": w_all[c],
        })
    try:
        res = run_bass_kernel_spmd(nc, in_maps, core_ids=list(range(NCORES)))
    except Exception:
        import time as _t
        _t.sleep(2.0)
        res = run_bass_kernel_spmd(nc, in_maps, core_ids=list(range(NCORES)))

    out = np.empty((5, N, D), np.float32)
    out[0] = x
    for c in range(NCORES):
        o = res.results[c]["out"]
        nodes = np.nonzero(node_core == c)[0]
        rk = rank[c, node_local[nodes]]
        out[1:, nodes, :] = o[:, rk, :]
    return out
